# revision 1
# baseline (speedup 1.0000x reference)
"""BiLSTM-CRF loss kernel for Trainium2 (8 NeuronCores, data-parallel over batch).

Design (per core, B_loc=16 sequences):
  - All state kept transposed: hidden dim on partitions, batch on free dim.
  - LSTM recurrence: weights-stationary matmuls (8 gate-chunks x 2 K-tiles,
    N=16 batch streaming), per-step masking via copy_predicated with a
    DMA-broadcast mask-replica tile.
  - Input projection x @ W_ih^T computed on the fly in 32-step windows
    (embedding gather -> PE transpose -> N=512 matmuls), never hits DRAM.
  - Emissions computed incrementally (2 small matmuls per step/direction)
    into a (20, T*16) SBUF buffer.
  - CRF log-partition via the *backward* (beta) recursion in exp space,
    folded into the backward-LSTM phase step by step; periodic per-column
    rescaling (compensated in log space) keeps fp32 in range.
  - Gold-path score: unary via host-built one-hot mask x emit reduce;
    transition term via indirect row-gather of `transition` by tags.
"""

import numpy as np

PAD_IDX = 0
VOCAB, K, E, H = 30000, 20, 256, 256
B, T = 128, 512
NCORES = 8
BL = B // NCORES          # 16 sequences per core
WIN = 32                  # proj window (time steps)
NW = T // WIN             # 16 windows
RESCALE = 8               # CRF rescale interval

_cache = {}


def _build_program(dt_w):
    """Build the SPMD Bass program. dt_w: matmul weight/stream dtype."""
    from contextlib import ExitStack
    import concourse.bass as bass
    import concourse.bacc as bacc
    import concourse.tile as tile
    from concourse import mybir
    from concourse.masks import make_identity

    f32 = mybir.dt.float32
    i32 = mybir.dt.int32

    nc = bacc.Bacc(None, target_bir_lowering=False, debug=False)
    names = {}

    with ExitStack() as ctx:
        tc = ctx.enter_context(tile.TileContext(nc))
        dram = ctx.enter_context(tc.tile_pool(name="dram", bufs=1, space="DRAM"))

        def din(key, shape, dt=f32):
            t = dram.tile(shape, dt, kind="ExternalInput", name=key)
            names[key] = t.tensor.name
            return t

        emb = din("emb", [VOCAB, E])
        toks = din("toks", [T * BL, 1], i32)          # window-major token ids
        maskf = din("maskf", [1, T * BL])             # col = t*16+b
        masku = din("masku", [1, T * BL], mybir.dt.uint8)
        tags1h = din("tags1h", [K, T * BL], mybir.dt.uint8)  # one-hot(tag) * mask
        tagsnx = din("tagsnx", [T * BL, K], mybir.dt.uint8)  # shifted one-hot * mask
        tagsfl = din("tagsfl", [T * BL, 1], i32)      # tag ids, b-major
        wih = {d: din(f"wih_{d}", [E, 4 * H], dt_w) for d in "fb"}
        whh = {d: din(f"whh_{d}", [E, 4 * H], dt_w) for d in "fb"}
        bih = {d: din(f"bih_{d}", [128, 8]) for d in "fb"}
        woutT = din("woutT", [4, 128, K], dt_w)       # chunks: Fk0,Fk1,Bk0,Bk1
        bout = din("bout", [K, 1])
        transT = din("transT", [K, K])                # transition.T
        trans = din("trans", [K, K])                  # raw, for row gather
        out_loss = dram.tile([1, BL], f32, kind="ExternalOutput")
        names["out"] = out_loss.tensor.name

        sg = ctx.enter_context(tc.tile_pool(name="sg", bufs=1))       # singles
        tmp = ctx.enter_context(tc.tile_pool(name="tmp", bufs=4))     # step temps
        gat = ctx.enter_context(tc.tile_pool(name="gat", bufs=8))     # gather tiles
        winp = ctx.enter_context(tc.tile_pool(name="winp", bufs=2))   # xw windows
        xtw = ctx.enter_context(tc.tile_pool(name="xtw", bufs=3))
        fin = ctx.enter_context(tc.tile_pool(name="fin", bufs=3))     # finalize
        ps_g = ctx.enter_context(tc.tile_pool(name="ps_g", bufs=2, space="PSUM"))
        ps_w = ctx.enter_context(tc.tile_pool(name="ps_w", bufs=1, space="PSUM"))
        ps_t = ctx.enter_context(tc.tile_pool(name="ps_t", bufs=1, space="PSUM"))
        ps_s = ctx.enter_context(tc.tile_pool(name="ps_s", bufs=4, space="PSUM"))

        # ---- resident SBUF tensors ----
        s_wih = {d: sg.tile([128, 2, 4 * H], dt_w, tag=f"wih{d}", name=f"wih{d}") for d in "fb"}
        s_whh = {d: sg.tile([128, 2, 4 * H], dt_w, tag=f"whh{d}", name=f"whh{d}") for d in "fb"}
        for d in "fb":
            nc.sync.dma_start(out=s_wih[d][:], in_=wih[d][:].rearrange("(k p) m -> p k m", p=128))
            nc.sync.dma_start(out=s_whh[d][:], in_=whh[d][:].rearrange("(k p) m -> p k m", p=128))
        s_bih = {d: sg.tile([128, 8], f32, tag=f"bih{d}", name=f"bih{d}") for d in "fb"}
        for d in "fb":
            nc.sync.dma_start(out=s_bih[d][:], in_=bih[d][:])
        s_wout = sg.tile([128, 4, K], dt_w, tag="wout")
        nc.sync.dma_start(out=s_wout[:], in_=woutT[:].rearrange("c p k -> p c k"))
        s_bout = sg.tile([K, 1], f32, tag="bout")
        nc.sync.dma_start(out=s_bout[:], in_=bout[:])
        s_transT = sg.tile([K, K], f32, tag="transT")
        nc.sync.dma_start(out=s_transT[:], in_=transT[:])
        s_expAT = sg.tile([K, K], f32, tag="expAT")
        nc.scalar.activation(s_expAT[:], s_transT[:], mybir.ActivationFunctionType.Exp)

        ones = sg.tile([128, K], f32, tag="ones")
        nc.vector.memset(ones[:], 1.0)
        ident = sg.tile([128, 128], f32, tag="ident")
        make_identity(nc, ident[:])

        # mask replica: (128, T, BL), col = t*16+b, broadcast across partitions
        maskrep = sg.tile([128, T, BL], mybir.dt.uint8, tag="maskrep")
        nc.sync.dma_start(
            out=maskrep[:],
            in_=bass.AP(tensor=masku.tensor, offset=masku[:].offset,
                        ap=[[0, 128], [BL, T], [1, BL]]),
        )
        maskrow = sg.tile([1, T, BL], f32, tag="maskrow")
        nc.sync.dma_start(out=maskrow[:],
                          in_=bass.AP(tensor=maskf.tensor, offset=maskf[:].offset,
                                      ap=[[0, 1], [BL, T], [1, BL]]))

        emit = sg.tile([K, T, BL], f32, tag="emit")

        # all gather indices resident (one upfront DMA each)
        NT128 = T * BL // 128
        idxall = sg.tile([128, NT128], i32, tag="idxall")
        nc.sync.dma_start(out=idxall[:],
                          in_=bass.AP(tensor=toks.tensor, offset=toks[:].offset,
                                      ap=[[1, 128], [128, NT128]]))
        idxtag = sg.tile([128, NT128], i32, tag="idxtag")
        nc.sync.dma_start(out=idxtag[:],
                          in_=bass.AP(tensor=tagsfl.tensor, offset=tagsfl[:].offset,
                                      ap=[[1, 128], [128, NT128]]))
        s_t1h = sg.tile([K, T, BL], mybir.dt.uint8, tag="s_t1h")
        nc.sync.dma_start(out=s_t1h[:].rearrange("k t b -> k (t b)"), in_=tags1h[:])
        s_tnx = sg.tile([128, NT128, K], mybir.dt.uint8, tag="s_tnx")
        nc.sync.dma_start(out=s_tnx[:],
                          in_=tagsnx[:].rearrange("(n p) k -> p n k", p=128))

        # LSTM states (h in dt_w for matmul rhs, c in f32)
        st_h = {d: sg.tile([128, 2, BL], dt_w, tag=f"h{d}", name=f"h{d}") for d in "fb"}
        st_c = {d: sg.tile([128, 2, BL], f32, tag=f"c{d}", name=f"c{d}") for d in "fb"}
        for d in "fb":
            nc.vector.memset(st_h[d][:], 0.0)
            nc.vector.memset(st_c[d][:], 0.0)

        # CRF beta state (exp space) + log-scale accumulator
        Bv = sg.tile([K, BL], f32, tag="Bv")
        nc.vector.memset(Bv[:], 1.0)
        Lacc = sg.tile([1, BL], f32, tag="Lacc")
        nc.vector.memset(Lacc[:], 0.0)

        AF = mybir.ActivationFunctionType
        OP = mybir.AluOpType

        NG, GB = 2, BL // 2

        def mask_ap(t, parts, reps, g=None):
            """maskrep[:parts, t, cols] replicated reps times along a middle dim."""
            cs = slice(g * GB, (g + 1) * GB) if g is not None else slice(0, BL)
            base = maskrep[0:parts, t, cs]
            if reps == 1:
                return base
            return bass.AP(tensor=base.tensor, offset=base.offset,
                           ap=[base.ap[0], [0, reps], [1, cs.stop - cs.start]])

        def make_window(w, d):
            """Gather+transpose+project window w for direction d.
            Returns xw window tile (128, 8, BL, WIN) with bias folded."""
            xT = xtw.tile([128, 2, 512], dt_w, tag="xT")
            for g in range(4):
                j = w * 4 + g
                xg = gat.tile([128, E], f32, tag="xg")
                nc.gpsimd.indirect_dma_start(
                    out=xg[:], out_offset=None, in_=emb[:],
                    in_offset=bass.IndirectOffsetOnAxis(ap=idxall[:, j:j + 1], axis=0),
                )
                xg2 = gat.tile([128, E], f32, tag="xg2")
                nc.vector.tensor_copy(xg2[:], xg[:])
                for k in range(2):
                    pst = ps_t.tile([128, 128], f32, tag="pst")
                    nc.tensor.transpose(out=pst[:], in_=xg2[:, k * 128:(k + 1) * 128], identity=ident[:])
                    nc.vector.tensor_copy(xT[:, k, g * 128:(g + 1) * 128], pst[:])
            win = winp.tile([128, 8, BL, WIN], f32, tag=f"win{d}", name=f"win{d}")
            for m in range(8):
                psw = ps_w.tile([128, 512], f32, tag="psw")
                for k in range(2):
                    nc.tensor.matmul(psw[:], lhsT=s_wih[d][:, k, m * 128:(m + 1) * 128],
                                     rhs=xT[:, k, :], start=(k == 0), stop=(k == 1))
                nc.vector.tensor_scalar_add(win[:, m], psw[:], s_bih[d][:, m:m + 1])
            return win

        def lstm_mm(d, t):
            """Full-width recurrence matmuls (both groups share LDWEIGHTS)."""
            h = st_h[d]
            psg = ps_g.tile([128, 8, BL], f32, tag="psg", name="psg", bufs=2)
            for m in range(8):
                for k in range(2):
                    nc.tensor.matmul(psg[:, m], lhsT=s_whh[d][:, k, m * 128:(m + 1) * 128],
                                     rhs=h[:, k, :], start=(k == 0), stop=(k == 1))
            return psg

        def lstm_stepA(d, t, win, psg, g):
            """Gate add + activations for group g."""
            cs = slice(g * GB, (g + 1) * GB)
            toff = t % WIN
            gates = tmp.tile([128, 8, GB], f32, tag=f"gates{g}", name=f"gates{g}")
            nc.vector.tensor_tensor(gates[:], psg[:, :, cs], win[:, :, cs, toff], op=OP.add)
            gf = gates[:].rearrange("p m b -> p (m b)")
            nc.scalar.activation(gf[:, 0:4 * GB], gf[:, 0:4 * GB], AF.Sigmoid)
            nc.scalar.activation(gf[:, 4 * GB:6 * GB], gf[:, 4 * GB:6 * GB], AF.Tanh)
            nc.scalar.activation(gf[:, 6 * GB:8 * GB], gf[:, 6 * GB:8 * GB], AF.Sigmoid)
            return gates

        def lstm_stepB(d, t, gates, emit_mode, g):
            """Cell update for group g."""
            cs = slice(g * GB, (g + 1) * GB)
            h, c = st_h[d][:, :, cs], st_c[d][:, :, cs]
            gi, gff, gg, go = (gates[:, 0:2], gates[:, 2:4], gates[:, 4:6], gates[:, 6:8])
            cc = tmp.tile([128, 2, GB], f32, tag=f"cc{g}", name=f"cc{g}")
            ig = tmp.tile([128, 2, GB], f32, tag=f"ig{g}", name=f"ig{g}")
            nc.gpsimd.tensor_tensor(ig[:], gi, gg, op=OP.mult)
            nc.vector.tensor_tensor(cc[:], gff, c, op=OP.mult)
            nc.vector.tensor_tensor(cc[:], cc[:], ig[:], op=OP.add)
            m2 = mask_ap(t, 128, 2, g)
            nc.vector.copy_predicated(c, m2, cc[:])
            th = tmp.tile([128, 2, GB], f32, tag=f"th{g}", name=f"th{g}")
            nc.scalar.activation(th[:], cc[:], AF.Tanh)
            hh = tmp.tile([128, 2, GB], dt_w, tag=f"hh{g}", name=f"hh{g}")
            nc.vector.tensor_tensor(hh[:], go, th[:], op=OP.mult)
            nc.vector.copy_predicated(h, m2, hh[:])

        def emit_step(d, t, emit_mode):
            h = st_h[d]
            pse = ps_s.tile([K, BL], f32, tag="pssm", name="pse")
            cbase = 0 if d == "f" else 2
            for k in range(2):
                nc.tensor.matmul(pse[:], lhsT=s_wout[:, cbase + k, :], rhs=h[:, k, :],
                                 start=(k == 0), stop=(k == 1))
            if emit_mode == "f":
                nc.vector.tensor_scalar_add(emit[:, t, :], pse[:], s_bout[:, 0:1])
            else:
                nc.vector.tensor_tensor(emit[:, t, :], pse[:], emit[:, t, :], op=OP.add)

        # warm-up matmuls: make PE's clock pass every weight-producing op so
        # steady-state matmuls carry at most one semaphore wait
        for wt in [s_wih["f"][:, 0, 0:1], s_wih["b"][:, 0, 0:1],
                   s_whh["f"][:, 0, 0:1], s_whh["b"][:, 0, 0:1],
                   s_wout[:, 0, 0:1]]:
            psd = ps_s.tile([1, 1], f32, tag="pssm")
            nc.tensor.matmul(psd[:], lhsT=wt, rhs=wt, start=True, stop=True)
        psd = ps_s.tile([1, 1], f32, tag="pssm")
        nc.tensor.matmul(psd[:], lhsT=s_expAT[0:K, 0:1], rhs=s_expAT[0:K, 0:1], start=True, stop=True)
        psd = ps_s.tile([1, 1], f32, tag="pssm")
        nc.tensor.matmul(psd[:], lhsT=ident[:, 0:1], rhs=ident[:, 0:1], start=True, stop=True)

        # ---------------- forward phase ----------------
        for w in range(NW):
            win = make_window(w, "f")
            for t in range(w * WIN, (w + 1) * WIN):
                psg = lstm_mm("f", t)
                gts = [lstm_stepA("f", t, win, psg, g) for g in range(NG)]
                for g in range(NG):
                    lstm_stepB("f", t, gts[g], "f", g)
                emit_step("f", t, "f")

        # ---------------- backward phase + CRF beta ----------------
        expE_prev = [None, None]
        for w in range(NW - 1, -1, -1):
            win = make_window(w, "b")
            for t in range((w + 1) * WIN - 1, w * WIN - 1, -1):
                psg = lstm_mm("b", t)
                gts = [lstm_stepA("b", t, win, psg, g) for g in range(NG)]
                for g in range(NG):
                    lstm_stepB("b", t, gts[g], "b", g)
                emit_step("b", t, "b")
                for g in range(NG):
                    cs = slice(g * GB, (g + 1) * GB)
                    expE = tmp.tile([K, GB], f32, tag=f"expE{g}", name=f"expE{g}")
                    nc.scalar.activation(expE[:], emit[:, t, cs], AF.Exp)
                    if t < T - 1:
                        bp = tmp.tile([K, GB], f32, tag=f"bp{g}", name=f"bp{g}")
                        nc.vector.tensor_tensor(bp[:], Bv[:, cs], expE_prev[g][:], op=OP.mult)
                        psb = ps_s.tile([K, GB], f32, tag="pssm", name="psb")
                        nc.tensor.matmul(psb[:], lhsT=s_expAT[:], rhs=bp[:], start=True, stop=True)
                        nc.vector.copy_predicated(Bv[:, cs], mask_ap(t + 1, K, 1, g), psb[:])
                    expE_prev[g] = expE
                if t < T - 1 and t % RESCALE == 0 and t > 0:
                    pss = ps_s.tile([1, BL], f32, tag="pssm", name="pss")
                    nc.tensor.matmul(pss[:], lhsT=ones[0:K, 0:1], rhs=Bv[:], start=True, stop=True)
                    rr = tmp.tile([1, BL], f32, tag="rr")
                    nc.vector.reciprocal(rr[:], pss[:])
                    psr = ps_s.tile([K, BL], f32, tag="pssm", name="psr")
                    nc.tensor.matmul(psr[:], lhsT=ones[0:1, 0:K], rhs=rr[:], start=True, stop=True)
                    sc = tmp.tile([K, BL], f32, tag="sc")
                    nc.vector.tensor_tensor(sc[:], Bv[:], psr[:], op=OP.mult)
                    nc.vector.copy_predicated(Bv[:], mask_ap(t, K, 1), sc[:])
                    lns = tmp.tile([1, BL], f32, tag="lns")
                    nc.scalar.activation(lns[:], pss[:], AF.Ln)
                    nc.vector.tensor_tensor(lns[:], lns[:], maskrow[0:1, t, :], op=OP.mult)
                    nc.vector.tensor_tensor(Lacc[:], Lacc[:], lns[:], op=OP.add)

        # ---------------- finalize ----------------
        # log partition: logZ = ln(sum_i expE_0 * Bv_0) + Lacc
        zt = fin.tile([K, BL], f32, tag="zt")
        for g in range(NG):
            cs = slice(g * GB, (g + 1) * GB)
            nc.vector.tensor_tensor(zt[:, cs], Bv[:, cs], expE_prev[g][:], op=OP.mult)
        psz = ps_s.tile([1, BL], f32, tag="pssm")
        nc.tensor.matmul(psz[:], lhsT=ones[0:K, 0:1], rhs=zt[:], start=True, stop=True)
        logZ = fin.tile([1, BL], f32, tag="logZ")
        nc.scalar.activation(logZ[:], psz[:], AF.Ln)
        nc.vector.tensor_tensor(logZ[:], logZ[:], Lacc[:], op=OP.add)

        # unary gold score: sum over (j,t) of tags1h * emit, keep b
        Uacc = fin.tile([K, BL], f32, tag="Uacc")
        nc.vector.memset(Uacc[:], 0.0)
        CH = 32
        TC = T // CH
        for ci in range(CH):
            t1 = fin.tile([K, TC * BL], f32, tag="t1")
            nc.vector.tensor_copy(t1[:], s_t1h[:, ci * TC:(ci + 1) * TC, :].rearrange("p t b -> p (t b)"))
            um = fin.tile([K, TC * BL], f32, tag="um")
            nc.vector.tensor_tensor(
                um[:], t1[:], emit[:, ci * TC:(ci + 1) * TC, :].rearrange("p t b -> p (t b)"),
                op=OP.mult)
            ur = fin.tile([K, BL], f32, tag="ur")
            umr = bass.AP(tensor=um.tensor, offset=um[:].offset,
                          ap=[um[:].ap[0], [1, BL], [BL, TC]])
            nc.vector.tensor_reduce(ur[:], umr, axis=mybir.AxisListType.X, op=OP.add)
            nc.vector.tensor_tensor(Uacc[:], Uacc[:], ur[:], op=OP.add)
        psu = ps_s.tile([1, BL], f32, tag="pssm")
        nc.tensor.matmul(psu[:], lhsT=ones[0:K, 0:1], rhs=Uacc[:], start=True, stop=True)
        score = fin.tile([1, BL], f32, tag="score")
        nc.vector.tensor_copy(score[:], psu[:])

        # transition gold score via row gather
        QT = T // 128
        TRbuf = fin.tile([128, NT128], f32, tag="TRbuf")
        for i in range(NT128):
            tr = gat.tile([128, K], f32, tag="tr")
            nc.gpsimd.indirect_dma_start(
                out=tr[:], out_offset=None, in_=trans[:],
                in_offset=bass.IndirectOffsetOnAxis(ap=idxtag[:, i:i + 1], axis=0))
            sel = gat.tile([128, K], f32, tag="sel")
            nc.vector.tensor_copy(sel[:], s_tnx[:, i, :])
            nc.vector.tensor_tensor(tr[:], tr[:], sel[:], op=OP.mult)
            nc.vector.tensor_reduce(TRbuf[:, i:i + 1], tr[:], axis=mybir.AxisListType.X, op=OP.add)
        pstr = ps_s.tile([1, NT128], f32, tag="pssm")
        nc.tensor.matmul(pstr[:], lhsT=ones[:, 0:1], rhs=TRbuf[:], start=True, stop=True)
        trv = fin.tile([1, BL], f32, tag="trv")
        ptr_ap = bass.AP(tensor=pstr.tensor, offset=pstr[:].offset,
                         ap=[pstr[:].ap[0], [QT, BL], [1, QT]])
        nc.vector.tensor_reduce(trv[:], ptr_ap, axis=mybir.AxisListType.X, op=OP.add)

        # loss = logZ - (score + trans)
        nc.vector.tensor_tensor(score[:], score[:], trv[:], op=OP.add)
        res = fin.tile([1, BL], f32, tag="res")
        nc.vector.tensor_tensor(res[:], logZ[:], score[:], op=OP.subtract)
        nc.sync.dma_start(out=out_loss[:], in_=res[:])

    nc.compile()
    return nc, names


def _prep_core(inputs, k, dt_np):
    """Build the per-core input map (host-side index plumbing only)."""
    s = slice(k * BL, (k + 1) * BL)
    sent = np.asarray(inputs["sentences"][s])          # (16, 512) i32
    tags = np.asarray(inputs["tags"][s])               # (16, 512) i32
    mask = (sent != PAD_IDX)
    maskf = mask.T.astype(np.float32).reshape(1, T * BL)       # col=t*16+b
    toks = sent.reshape(BL, NW, WIN).transpose(1, 0, 2).reshape(T * BL, 1)
    oh = (tags[:, :, None] == np.arange(K)[None, None, :])
    tags1h = (oh & mask[:, :, None]).transpose(2, 1, 0).reshape(K, T * BL)
    tnx = np.zeros((BL, T, K), np.float32)
    tnx[:, :-1, :] = (oh[:, 1:, :] & mask[:, 1:, None]).astype(np.float32)
    m = {
        "toks": toks.astype(np.int32),
        "maskf": maskf,
        "masku": mask.T.astype(np.uint8).reshape(1, T * BL),
        "tags1h": tags1h.astype(np.uint8),
        "tagsnx": tnx.reshape(T * BL, K).astype(np.uint8),
        "tagsfl": tags.reshape(T * BL, 1).astype(np.int32),
        "emb": np.asarray(inputs["embedding"], np.float32),
        "wih_f": np.ascontiguousarray(np.asarray(inputs["w_ih_f"]).T).astype(dt_np),
        "wih_b": np.ascontiguousarray(np.asarray(inputs["w_ih_b"]).T).astype(dt_np),
        "whh_f": np.ascontiguousarray(np.asarray(inputs["w_hh_f"]).T).astype(dt_np),
        "whh_b": np.ascontiguousarray(np.asarray(inputs["w_hh_b"]).T).astype(dt_np),
        "bih_f": np.ascontiguousarray(np.asarray(inputs["b_f"]).reshape(8, 128).T).astype(np.float32),
        "bih_b": np.ascontiguousarray(np.asarray(inputs["b_b"]).reshape(8, 128).T).astype(np.float32),
        "woutT": np.ascontiguousarray(np.asarray(inputs["w_out"]).T.reshape(4, 128, K)).astype(dt_np),
        "bout": np.asarray(inputs["b_out"]).reshape(K, 1).astype(np.float32),
        "transT": np.ascontiguousarray(np.asarray(inputs["transition"]).T).astype(np.float32),
        "trans": np.asarray(inputs["transition"], np.float32),
    }
    return m


def kernel(**inputs):
    import ml_dtypes
    from concourse import mybir
    from concourse.bass_utils import run_bass_kernel_spmd

    use_bf16 = _cache.get("use_bf16", True)
    key = ("prog", use_bf16)
    if key not in _cache:
        dt_w = mybir.dt.bfloat16 if use_bf16 else mybir.dt.float32
        _cache[key] = _build_program(dt_w)
    nc, names = _cache[key]
    dt_np = ml_dtypes.bfloat16 if use_bf16 else np.float32

    in_maps = []
    for k in range(NCORES):
        m = _prep_core(inputs, k, dt_np)
        in_maps.append({names[kk]: vv for kk, vv in m.items()})

    res = run_bass_kernel_spmd(nc, in_maps, core_ids=list(range(NCORES)),
                               **_cache.get("run_kwargs", {}))
    out = np.concatenate([r[names["out"]].reshape(BL) for r in res.results])
    _cache["last_results"] = res
    return out.astype(np.float32)



# revision 13
# speedup vs baseline: 1.9781x; 1.9781x over previous
"""BiLSTM-CRF loss kernel for Trainium2 (8 NeuronCores, data-parallel over batch).

Design (per core, B_loc=16 sequences), v2:
  - Hidden dim on partitions, batch on free dim everywhere.
  - Input projection xw = x @ W_ih^T computed per 8-step window directly
    into a PSUM slot (2 slots, 2 banks each, ping-pong); gate bias and
    (bwd only) the -1e5 pad-kill term are folded in as rank-1 matmuls into
    the same slot.  Window builds are streamed piecewise into the step
    loop so the PE never bursts at window boundaries.
  - Per-step recurrence matmuls ACCUMULATE onto the slot columns
    (start=False), so gate assembly never touches the vector engines.
  - Activations: ONE sigmoid over all 8 gate chunks (g-rows pre-doubled on
    host; tanh(x) = 2*sigmoid(2x)-1 fixed up in the cell math) + ONE
    tanh(c) per group.  Sigmoid and tanh share an activation table set, so
    the LSTM loop never reloads activation tables.
  - Forward direction runs fully unmasked (pad is a suffix; its garbage
    state is tanh-bounded and never observed).  Backward masking comes from
    the -1e5 gate injection (i=f=o=0 => h=c=0 exactly).
  - CRF beta recursion (exp space) folded into the backward phase, lagged
    ~1 block; exp(emit) batched per 32-step block (2 act-table swaps per
    32 steps).  Rescaling every 8 beta steps extracts the fp32 exponent
    with integer ALU ops (no Ln, no reciprocal) and compensates at the end.
  - Gold-path unary score accumulated per 32-step block on GpSimd;
    transition score via indirect row-gathers spread over phase 1.
"""

import numpy as np

PAD_IDX = 0
VOCAB, K, E, H = 30000, 20, 256, 256
B, T = 128, 512
NCORES = 8
BL = B // NCORES          # 16 sequences per core
NG = 2                    # batch groups for chain overlap
GB = BL // NG
SLOTW = 8                 # window length (steps); slot chunk = 512B
NW = T // SLOTW           # 64 windows
BLK = 32                  # CRF exp/unary block size
RESCALE = 8               # CRF rescale interval (beta steps)
NCH = 64                  # 128-token gather chunks (T*BL/128)

_cache = {}


def _build_program():
    from contextlib import ExitStack
    import concourse.bass as bass
    import concourse.bacc as bacc
    import concourse.tile as tile
    from concourse import mybir
    from concourse.masks import make_identity

    f32 = mybir.dt.float32
    i32 = mybir.dt.int32
    bf16 = mybir.dt.bfloat16
    u8 = mybir.dt.uint8
    AF = mybir.ActivationFunctionType
    OP = mybir.AluOpType

    nc = bacc.Bacc(None, target_bir_lowering=False, debug=False)
    names = {}

    with ExitStack() as ctx:
        tc = ctx.enter_context(tile.TileContext(nc))
        dram = ctx.enter_context(tc.tile_pool(name="dram", bufs=1, space="DRAM"))

        def din(key, shape, dt=f32):
            t = dram.tile(shape, dt, kind="ExternalInput", name=key)
            names[key] = t.tensor.name
            return t

        emb = din("emb", [VOCAB, E], bf16)
        toks = din("toks", [T * BL, 1], i32)            # (t,b) order token ids
        masku = din("masku", [1, T * BL], u8)           # col = t*16+b
        negm = din("negm", [1, T * BL], bf16)           # (1-m) * -1e5
        tags1f = din("tags1f", [K, T * BL], f32)        # one-hot(tag)*mask, f32
        tagsnx = din("tagsnx", [T * BL, K], u8)         # shifted one-hot*mask
        tagsfl = din("tagsfl", [T * BL, 1], i32)        # tag ids
        wih = {d: din(f"wih_{d}", [E, 4 * H], bf16) for d in "fb"}
        whh = {d: din(f"whh_{d}", [E, 4 * H], bf16) for d in "fb"}
        brow = {d: din(f"brow_{d}", [1, 4 * H], bf16) for d in "fb"}
        woutT = din("woutT", [4, 128, K], bf16)         # chunks: Fk0,Fk1,Bk0,Bk1
        bout = din("bout", [K, 1])
        transT = din("transT", [K, K])                  # transition.T
        trans = din("trans", [K, K])                    # raw, for row gather
        out_loss = dram.tile([1, BL], f32, kind="ExternalOutput")
        names["out"] = out_loss.tensor.name
        out_dbg = dram.tile([1, 4 * BL], f32, kind="ExternalOutput", name="out_dbg")
        names["dbg"] = out_dbg.tensor.name

        # PSUM pools first so the window slots land in the low banks
        ps_slot = ctx.enter_context(tc.tile_pool(name="ps_slot", bufs=1, space="PSUM"))
        ps_t = ctx.enter_context(tc.tile_pool(name="ps_t", bufs=1, space="PSUM"))
        ps_s = ctx.enter_context(tc.tile_pool(name="ps_s", bufs=3, space="PSUM"))

        sg = ctx.enter_context(tc.tile_pool(name="sg", bufs=1))
        tmp = ctx.enter_context(tc.tile_pool(name="tmp", bufs=4))
        gat = ctx.enter_context(tc.tile_pool(name="gat", bufs=4))
        neg = ctx.enter_context(tc.tile_pool(name="neg", bufs=2))
        fin = ctx.enter_context(tc.tile_pool(name="fin", bufs=3))

        # PSUM window slots: [128, 8, 8, 16] fp32 = 4KB/partition (2 banks)
        slot = [ps_slot.tile([128, 8, SLOTW, BL], f32, tag=f"slot{i}",
                             name=f"slot{i}") for i in range(2)]

        # ---- resident SBUF tensors ----
        s_wih = {d: sg.tile([128, 2, 4 * H], bf16, tag=f"wih{d}", name=f"wih{d}")
                 for d in "fb"}
        s_whh = {d: sg.tile([128, 2, 4 * H], bf16, tag=f"whh{d}", name=f"whh{d}")
                 for d in "fb"}
        s_brow = {d: sg.tile([1, 4 * H], bf16, tag=f"brow{d}", name=f"brow{d}")
                  for d in "fb"}
        for d in "fb":
            nc.sync.dma_start(out=s_wih[d][:], in_=wih[d][:].rearrange("(k p) m -> p k m", p=128))
            nc.sync.dma_start(out=s_whh[d][:], in_=whh[d][:].rearrange("(k p) m -> p k m", p=128))
            nc.sync.dma_start(out=s_brow[d][:], in_=brow[d][:])
        s_wout = sg.tile([128, 4, K], bf16, tag="wout")
        nc.sync.dma_start(out=s_wout[:], in_=woutT[:].rearrange("c p k -> p c k"))
        s_bout = sg.tile([K, 1], f32, tag="bout")
        nc.sync.dma_start(out=s_bout[:], in_=bout[:])
        s_transT = sg.tile([K, K], f32, tag="transT")
        nc.sync.dma_start(out=s_transT[:], in_=transT[:])
        s_expAT = sg.tile([K, K], f32, tag="expAT")
        nc.scalar.activation(s_expAT[:], s_transT[:], AF.Exp)

        onesb = sg.tile([1, 256], bf16, tag="onesb")    # rank-1 rhs for bias mm
        nc.vector.memset(onesb[:], 1.0)
        onesrow = sg.tile([1, 128], bf16, tag="onesrow")  # rank-1 lhsT for pad mm
        nc.vector.memset(onesrow[:], 1.0)
        onesf = sg.tile([128, K], f32, tag="onesf")
        nc.vector.memset(onesf[:], 1.0)
        ident = sg.tile([128, 128], bf16, tag="ident")
        make_identity(nc, ident[:])

        c_mask = sg.tile([1, 1], i32, tag="c_mask")
        nc.vector.memset(c_mask[:], 0x7F800000)
        c_sub = sg.tile([1, 1], i32, tag="c_sub")
        nc.vector.memset(c_sub[:], 0x7F000000)
        c_negone = sg.tile([1, 1], i32, tag="c_negone")
        nc.vector.memset(c_negone[:], -1)

        maskrep = sg.tile([128, T, BL], u8, tag="maskrep")
        nc.sync.dma_start(
            out=maskrep[:],
            in_=bass.AP(tensor=masku.tensor, offset=masku[:].offset,
                        ap=[[0, 128], [BL, T], [1, BL]]),
        )

        idxall = sg.tile([128, NCH], i32, tag="idxall")
        nc.sync.dma_start(out=idxall[:],
                          in_=bass.AP(tensor=toks.tensor, offset=toks[:].offset,
                                      ap=[[1, 128], [128, NCH]]))
        idxtag = sg.tile([128, NCH], i32, tag="idxtag")
        nc.sync.dma_start(out=idxtag[:],
                          in_=bass.AP(tensor=tagsfl.tensor, offset=tagsfl[:].offset,
                                      ap=[[1, 128], [128, NCH]]))
        s_t1f = sg.tile([K, T * BL], f32, tag="s_t1f")
        nc.sync.dma_start(out=s_t1f[:], in_=tags1f[:])
        s_tnx = sg.tile([128, NCH, K], u8, tag="s_tnx")
        nc.sync.dma_start(out=s_tnx[:],
                          in_=tagsnx[:].rearrange("(n p) k -> p n k", p=128))

        xT = sg.tile([128, 2, T * BL], bf16, tag="xT")
        emit = sg.tile([K, T, BL], f32, tag="emit")
        expE = sg.tile([K, 2, BLK * BL], f32, tag="expE")

        st_h = {d: sg.tile([128, 2, BL], bf16, tag=f"h{d}", name=f"h{d}") for d in "fb"}
        st_c = {d: sg.tile([128, 2, BL], f32, tag=f"c{d}", name=f"c{d}") for d in "fb"}
        for d in "fb":
            nc.vector.memset(st_h[d][:], 0.0)
            nc.vector.memset(st_c[d][:], 0.0)

        Bv = sg.tile([K, BL], f32, tag="Bv")
        nc.vector.memset(Bv[:], 1.0)
        Eacc = sg.tile([1, BL], f32, tag="Eacc")
        nc.vector.memset(Eacc[:], 0.0)
        Uacc = sg.tile([K, BL], f32, tag="Uacc")
        nc.vector.memset(Uacc[:], 0.0)
        TRbuf = sg.tile([128, NCH], f32, tag="TRbuf")

        # ---- warm-up matmuls ----
        for wt in [s_wih["f"][:, 0, 0:1], s_wih["b"][:, 0, 0:1],
                   s_whh["f"][:, 0, 0:1], s_whh["b"][:, 0, 0:1],
                   s_wout[:, 0, 0:1], ident[:, 0:1]]:
            psd = ps_s.tile([1, 1], f32, tag="pssm", name="psd")
            nc.tensor.matmul(psd[:], lhsT=wt, rhs=wt, start=True, stop=True)
        psd = ps_s.tile([1, 1], f32, tag="pssm", name="psd")
        nc.tensor.matmul(psd[:], lhsT=s_expAT[0:K, 0:1], rhs=s_expAT[0:K, 0:1],
                         start=True, stop=True)

        # ---------------- background work (streamed into step loops) --------
        bg_q = []

        def drain_bg(n):
            while n > 0 and bg_q:
                bg_q.pop(0)()
                n -= 1

        gathered = [0]

        def gather_chunk(c):
            def work():
                xg = gat.tile([128, E], bf16, tag="xg", name="xg")
                nc.gpsimd.indirect_dma_start(
                    out=xg[:], out_offset=None, in_=emb[:],
                    in_offset=bass.IndirectOffsetOnAxis(ap=idxall[:, c:c + 1], axis=0),
                )
                for k in range(2):
                    pst = ps_t.tile([128, 128], bf16, tag="pst", name="pst")
                    nc.tensor.transpose(out=pst[:], in_=xg[:, k * 128:(k + 1) * 128],
                                        identity=ident[:])
                    nc.vector.tensor_copy(xT[:, k, c * 128:(c + 1) * 128], pst[:])
            return work

        def tag_chunk(i):
            def work():
                tr = gat.tile([128, K], f32, tag="tr", name="tr")
                nc.gpsimd.indirect_dma_start(
                    out=tr[:], out_offset=None, in_=trans[:],
                    in_offset=bass.IndirectOffsetOnAxis(ap=idxtag[:, i:i + 1], axis=0))
                sel = gat.tile([128, K], f32, tag="sel", name="sel")
                nc.vector.tensor_copy(sel[:], s_tnx[:, i, :])
                nc.vector.tensor_tensor(tr[:], tr[:], sel[:], op=OP.mult)
                nc.vector.tensor_reduce(TRbuf[:, i:i + 1], tr[:],
                                        axis=mybir.AxisListType.X, op=OP.add)
            return work

        # ---------------- window build pieces ----------------
        def negdma_piece(w, nm):
            c0, ncol = w * SLOTW * BL, SLOTW * BL

            def work():
                nc.sync.dma_start(out=nm[:, 0:ncol], in_=negm[:, c0:c0 + ncol])
            return work

        def proj_piece(w, d, m0, nm):
            """Project chunks m0, m0+1 of window w into the slot."""
            c0, ncol = w * SLOTW * BL, SLOTW * BL
            sl = slot[w % 2]

            def work():
                for m in (m0, m0 + 1):
                    o_m = bass.AP(tensor=sl.tensor,
                                  offset=sl[:].offset + m * SLOTW * BL,
                                  ap=[sl[:].ap[0], [1, ncol]])
                    for k in range(2):
                        # start=True only on the first matmul touching each
                        # psum bank (chunks 0-3 -> bank A, 4-7 -> bank B)
                        st = (m in (0, 4)) and k == 0
                        nc.tensor.matmul(o_m, lhsT=s_wih[d][:, k, m * 128:(m + 1) * 128],
                                         rhs=xT[:, k, c0:c0 + ncol],
                                         start=st, stop=False,
                                         skip_group_check=True)
                    nc.tensor.matmul(o_m, lhsT=s_brow[d][:, m * 128:(m + 1) * 128],
                                     rhs=onesb[:, 0:ncol], start=False, stop=False,
                                     skip_group_check=True)
                    if d == "b" and m < 6:
                        nc.tensor.matmul(o_m, lhsT=onesrow[:, 0:128],
                                         rhs=nm[:, 0:ncol], start=False, stop=False,
                                         skip_group_check=True)
            return work

        def build_pieces(w, d):
            pieces = []
            nm = None
            if d == "b":
                nm = neg.tile([1, 256], bf16, tag="nm", name="nm")
                pieces.append(negdma_piece(w, nm))
            for m0 in range(0, 8, 2):
                pieces.append(proj_piece(w, d, m0, nm))
            return pieces

        # ---------------- LSTM step ----------------
        def lstm_step(d, t, w):
            sl = slot[w % 2]
            toff = t - w * SLOTW
            h, c = st_h[d], st_c[d]
            for m in range(8):
                o_m = bass.AP(tensor=sl.tensor,
                              offset=sl[:].offset + (m * SLOTW + toff) * BL,
                              ap=[sl[:].ap[0], [1, BL]])
                for k in range(2):
                    nc.tensor.matmul(o_m, lhsT=s_whh[d][:, k, m * 128:(m + 1) * 128],
                                     rhs=h[:, k, :], start=False, stop=False,
                                     skip_group_check=True)
            for g in range(NG):
                cs = slice(g * GB, (g + 1) * GB)
                gin = bass.AP(tensor=sl.tensor,
                              offset=sl[:].offset + toff * BL + g * GB,
                              ap=[sl[:].ap[0], [SLOTW * BL, 8], [1, GB]])
                s = tmp.tile([128, 8, GB], f32, tag=f"s{g}", name=f"s{g}")
                nc.scalar.activation(s[:], gin, AF.Sigmoid)
                si, sf, so, sgt = s[:, 0:2], s[:, 2:4], s[:, 4:6], s[:, 6:8]
                ig = tmp.tile([128, 2, GB], f32, tag=f"ig{g}", name=f"ig{g}")
                nc.vector.scalar_tensor_tensor(ig[:], sgt, 0.5, si,
                                               op0=OP.subtract, op1=OP.mult)
                fc = tmp.tile([128, 2, GB], f32, tag=f"fc{g}", name=f"fc{g}")
                nc.vector.tensor_tensor(fc[:], sf, c[:, :, cs], op=OP.mult)
                nc.vector.scalar_tensor_tensor(c[:, :, cs], ig[:], 2.0, fc[:],
                                               op0=OP.mult, op1=OP.add)
                th = tmp.tile([128, 2, GB], f32, tag=f"th{g}", name=f"th{g}")
                nc.scalar.activation(th[:], c[:, :, cs], AF.Tanh)
                nc.vector.tensor_tensor(h[:, :, cs], so, th[:], op=OP.mult)

        def emit_step(d, t):
            h = st_h[d]
            pse = ps_s.tile([K, BL], f32, tag="pssm", name="pse")
            cbase = 0 if d == "f" else 2
            for k in range(2):
                nc.tensor.matmul(pse[:], lhsT=s_wout[:, cbase + k, :], rhs=h[:, k, :],
                                 start=(k == 0), stop=(k == 1))
            if d == "f":
                # emit = pse + b_out on the scalar engine (Identity + bias)
                nc.scalar.activation(emit[:, t, :], pse[:], AF.Identity,
                                     bias=s_bout[:, 0:1])
            else:
                nc.vector.tensor_tensor(emit[:, t, :], pse[:], emit[:, t, :], op=OP.add)

        # ---------------- CRF beta machinery ----------------
        rescale_count = [0]

        def exp_piece(blk):
            def work():
                src = emit[:, blk * BLK:(blk + 1) * BLK, :].rearrange("k t b -> k (t b)")
                nc.scalar.activation(expE[:, blk % 2, :], src, AF.Exp)
            return work

        def unary_piece(blk):
            c0 = blk * BLK * BL
            n = BLK * BL

            def work():
                src = emit[:, blk * BLK:(blk + 1) * BLK, :].rearrange("k t b -> k (t b)")
                um = tmp.tile([K, n], f32, tag="um", name="um")
                nc.gpsimd.tensor_tensor(um[:], s_t1f[:, c0:c0 + n], src, op=OP.mult)
                ur = tmp.tile([K, BL], f32, tag="ur", name="ur")
                umr = bass.AP(tensor=um.tensor, offset=um[:].offset,
                              ap=[um[:].ap[0], [1, BL], [BL, BLK]])
                nc.vector.tensor_reduce(ur[:], umr, axis=mybir.AxisListType.X, op=OP.add)
                nc.vector.tensor_tensor(Uacc[:], Uacc[:], ur[:], op=OP.add)
            return work

        def beta_step(s):
            def work():
                blk = (s + 1) // BLK
                col = ((s + 1) % BLK) * BL
                bp = tmp.tile([K, BL], f32, tag="bp", name="bp")
                nc.gpsimd.tensor_tensor(bp[:], Bv[:], expE[:, blk % 2, col:col + BL],
                                        op=OP.mult)
                psb = ps_s.tile([K, BL], f32, tag="pssm", name="psb")
                nc.tensor.matmul(psb[:], lhsT=s_expAT[:], rhs=bp[:], start=True, stop=True)
                nc.vector.copy_predicated(Bv[:], maskrep[0:K, s + 1, :], psb[:])
            return work

        def beta_rescale():
            def work():
                pss = ps_s.tile([1, BL], f32, tag="pssm", name="pssr")
                nc.tensor.matmul(pss[:], lhsT=onesf[0:K, 0:1], rhs=Bv[:],
                                 start=True, stop=True)
                em = tmp.tile([1, BL], i32, tag="em", name="em")
                nc.vector.tensor_scalar(em[:], pss[:].bitcast(i32), c_mask[:, 0:1],
                                        None, op0=OP.bitwise_and)
                ef = tmp.tile([1, BL], f32, tag="ef", name="ef")
                nc.vector.tensor_copy(ef[:], em[:])
                nc.vector.scalar_tensor_tensor(Eacc[:], ef[:], 1.0 / (1 << 23), Eacc[:],
                                               op0=OP.mult, op1=OP.add)
                # scale bits = 0x7F000000 - em, computed exactly in f32
                scf = tmp.tile([1, BL], f32, tag="scf", name="scf")
                nc.vector.tensor_scalar(scf[:], ef[:], -1.0, float(0x7F000000),
                                        op0=OP.mult, op1=OP.add)
                sci = tmp.tile([1, BL], i32, tag="sci", name="sci")
                nc.vector.tensor_copy(sci[:], scf[:])
                psr = ps_s.tile([K, BL], f32, tag="pssm", name="psr")
                nc.tensor.matmul(psr[:], lhsT=onesf[0:1, 0:K], rhs=sci[:].bitcast(f32),
                                 start=True, stop=True)
                nc.vector.tensor_tensor(Bv[:], Bv[:], psr[:], op=OP.mult)
            rescale_count[0] += 1
            return work

        beta_q = []

        def drain_beta(n):
            while n > 0 and beta_q:
                beta_q.pop(0)()
                n -= 1

        def enqueue_block(blk):
            beta_q.append(exp_piece(blk))
            beta_q.append(unary_piece(blk))
            hi = blk * BLK + BLK - 1 if blk < T // BLK - 1 else T - 2
            lo = blk * BLK - 1
            for s in range(hi, lo, -1):
                beta_q.append(beta_step(s))
                if s % RESCALE == 0 and s > 0:
                    beta_q.append(beta_rescale())

        # ================ phase 1: forward LSTM ================
        def enqueue_tokens(upto_col):
            while gathered[0] * 128 < upto_col and gathered[0] < NCH:
                bg_q.append(gather_chunk(gathered[0]))
                gathered[0] += 1

        enqueue_tokens(SLOTW * BL)
        for p in build_pieces(0, "f"):
            bg_q.append(p)
        drain_bg(len(bg_q))  # prime window 0 eagerly
        tag_i = [0]
        for w in range(NW):
            if w + 1 < NW:
                enqueue_tokens((w + 2) * SLOTW * BL)
                for p in build_pieces(w + 1, "f"):
                    bg_q.append(p)
            if tag_i[0] < NCH:
                bg_q.append(tag_chunk(tag_i[0]))
                tag_i[0] += 1
            for t in range(w * SLOTW, (w + 1) * SLOTW):
                lstm_step("f", t, w)
                emit_step("f", t)
                drain_bg(1)
            if w + 1 < NW:
                drain_bg(len(bg_q))
        drain_bg(len(bg_q))

        # ================ phase 2: backward LSTM + CRF beta ================
        for p in build_pieces(NW - 1, "b"):
            bg_q.append(p)
        drain_bg(len(bg_q))
        for w in range(NW - 1, -1, -1):
            if w > 0:
                for p in build_pieces(w - 1, "b"):
                    bg_q.append(p)
            for t in range((w + 1) * SLOTW - 1, w * SLOTW - 1, -1):
                lstm_step("b", t, w)
                emit_step("b", t)
                if t % BLK == 0:
                    enqueue_block(t // BLK)
                drain_bg(1)
                drain_beta(2)
            if w > 0:
                drain_bg(len(bg_q))
        drain_beta(len(beta_q))

        # ================ finalize ================
        zt = fin.tile([K, BL], f32, tag="zt")
        nc.vector.tensor_tensor(zt[:], Bv[:], expE[:, 0, 0:BL], op=OP.mult)
        psz = ps_s.tile([1, BL], f32, tag="pssm", name="psz")
        nc.tensor.matmul(psz[:], lhsT=onesf[0:K, 0:1], rhs=zt[:], start=True, stop=True)
        logZ = fin.tile([1, BL], f32, tag="logZ")
        nc.scalar.activation(logZ[:], psz[:], AF.Ln)
        nc.vector.scalar_tensor_tensor(logZ[:], Eacc[:], float(np.log(2.0)), logZ[:],
                                       op0=OP.mult, op1=OP.add)
        nc.vector.tensor_scalar(logZ[:], logZ[:],
                                float(-127.0 * rescale_count[0] * np.log(2.0)), None,
                                op0=OP.add)

        psu = ps_s.tile([1, BL], f32, tag="pssm", name="psu")
        nc.tensor.matmul(psu[:], lhsT=onesf[0:K, 0:1], rhs=Uacc[:], start=True, stop=True)
        score = fin.tile([1, BL], f32, tag="score")
        nc.vector.tensor_copy(score[:], psu[:])

        QT = T // 128
        pstr = ps_s.tile([1, NCH], f32, tag="pssm", name="pstr")
        nc.tensor.matmul(pstr[:], lhsT=onesf[:, 0:1], rhs=TRbuf[:], start=True, stop=True)
        trv = fin.tile([1, BL], f32, tag="trv")
        ptr_ap = bass.AP(tensor=pstr.tensor, offset=pstr[:].offset,
                         ap=[pstr[:].ap[0], [QT, BL], [1, QT]])
        nc.vector.tensor_reduce(trv[:], ptr_ap, axis=mybir.AxisListType.X, op=OP.add)

        dbg = fin.tile([1, 4 * BL], f32, tag="dbg")
        nc.vector.tensor_copy(dbg[:, 0 * BL:1 * BL], score[:])   # unary
        nc.vector.tensor_copy(dbg[:, 1 * BL:2 * BL], trv[:])     # trans
        nc.vector.tensor_copy(dbg[:, 2 * BL:3 * BL], logZ[:])    # logZ
        nc.vector.tensor_copy(dbg[:, 3 * BL:4 * BL], Eacc[:])    # rescale exps
        nc.sync.dma_start(out=out_dbg[:], in_=dbg[:])

        nc.vector.tensor_tensor(score[:], score[:], trv[:], op=OP.add)
        res = fin.tile([1, BL], f32, tag="res")
        nc.vector.tensor_tensor(res[:], logZ[:], score[:], op=OP.subtract)
        nc.sync.dma_start(out=out_loss[:], in_=res[:])

    nc.compile()
    return nc, names


def _prep_core(inputs, core, perm):
    """Per-core input map. perm: gate row permutation to (i,f,o,g) chunk order."""
    import ml_dtypes
    bf = ml_dtypes.bfloat16
    s = slice(core * BL, (core + 1) * BL)
    sent = np.asarray(inputs["sentences"][s])
    tags = np.asarray(inputs["tags"][s])
    mask = (sent != PAD_IDX)
    maskT = mask.T                                     # (t, b)
    toks = np.ascontiguousarray(sent.T).reshape(T * BL, 1)
    oh = (tags[:, :, None] == np.arange(K)[None, None, :])
    tags1h = (oh & mask[:, :, None]).transpose(2, 1, 0).reshape(K, T * BL)
    tnx = np.zeros((BL, T, K), np.float32)
    tnx[:, :-1, :] = (oh[:, 1:, :] & mask[:, 1:, None]).astype(np.float32)

    def wprep(wname):
        wt = np.asarray(inputs[wname], np.float32)[perm].copy()
        wt[6 * 128:, :] *= 2.0                         # double g rows
        return np.ascontiguousarray(wt.T).astype(bf)

    bvec = {}
    for d, key in (("f", "b_f"), ("b", "b_b")):
        bb = np.asarray(inputs[key], np.float32)[perm].copy()
        bb[6 * 128:] *= 2.0
        bvec[d] = bb.reshape(1, 4 * H).astype(bf)

    return {
        "toks": toks.astype(np.int32),
        "masku": maskT.astype(np.uint8).reshape(1, T * BL),
        "negm": ((~maskT).astype(np.float32) * -1e5).reshape(1, T * BL).astype(bf),
        "tags1f": tags1h.astype(np.float32),
        "tagsnx": tnx.reshape(T * BL, K).astype(np.uint8),
        "tagsfl": tags.reshape(T * BL, 1).astype(np.int32),
        "emb": np.asarray(inputs["embedding"], np.float32).astype(bf),
        "wih_f": wprep("w_ih_f"), "wih_b": wprep("w_ih_b"),
        "whh_f": wprep("w_hh_f"), "whh_b": wprep("w_hh_b"),
        "brow_f": bvec["f"], "brow_b": bvec["b"],
        "woutT": np.ascontiguousarray(
            np.asarray(inputs["w_out"], np.float32).T.reshape(4, 128, K)).astype(bf),
        "bout": np.asarray(inputs["b_out"]).reshape(K, 1).astype(np.float32),
        "transT": np.ascontiguousarray(np.asarray(inputs["transition"]).T).astype(np.float32),
        "trans": np.asarray(inputs["transition"], np.float32),
    }


def kernel(**inputs):
    from concourse.bass_utils import run_bass_kernel_spmd

    if "prog" not in _cache:
        _cache["prog"] = _build_program()
    nc, names = _cache["prog"]

    # torch gate order i,f,g,o (H rows each) -> chunk order i,f,o,g
    blocks = np.arange(4 * H).reshape(4, H)
    perm = np.concatenate([blocks[0], blocks[1], blocks[3], blocks[2]])

    in_maps = []
    for core in range(NCORES):
        m = _prep_core(inputs, core, perm)
        in_maps.append({names[kk]: vv for kk, vv in m.items()})

    res = run_bass_kernel_spmd(nc, in_maps, core_ids=list(range(NCORES)),
                               **_cache.get("run_kwargs", {}))
    out = np.concatenate([r[names["out"]].reshape(BL) for r in res.results])
    _cache["last_results"] = res
    if "dbg" in names:
        _cache["dbg"] = np.concatenate(
            [r[names["dbg"]].reshape(4, BL) for r in res.results], axis=1)
    return out.astype(np.float32)


# revision 16
# speedup vs baseline: 2.2252x; 1.1249x over previous
"""BiLSTM-CRF loss kernel for Trainium2 (8 NeuronCores, data-parallel over batch).

v3: merged-direction LSTM phase + separate pipelined CRF tail.
  - Both LSTM directions run interleaved in ONE loop: two independent
    dependency chains pipeline across PE/Act/DVE/Pool, hiding the per-step
    serial latency that bounded v2.
  - Hidden state written straight into per-direction h histories (bf16);
    emissions are assembled in batched 32-step blocks (4 matmuls + one
    scalar-engine bias pass per 128-col chunk) once both directions have
    covered the block - no per-step emission work at all.
  - Input projection per 4-step window into 1-bank PSUM slots (2 per
    direction, ping-pong); gate bias and the bwd -1e5 pad-kill folded in as
    rank-1 matmuls; recurrence matmuls accumulate in place (start=False).
  - Activations: one sigmoid over all 8 gate chunks per group (g-rows
    pre-doubled; tanh(x)=2*sigmoid(2x)-1 fixed in cell math) + tanh(c) per
    group; sigmoid+tanh share one act table -> no table loads in the loop.
  - Forward direction unmasked (pad suffix garbage is bounded and never
    read); backward masked via the -1e5 gate injection (h=c=0 exactly).
  - CRF beta recursion in exp space as a tail, 2 column-subgroups
    pipelined; exp(emit) batched per block (sigmoid never used in the tail
    -> one act-table load total); rescale every 8 steps via fp32 exponent
    bit extraction (no Ln / reciprocal).
"""

import numpy as np

PAD_IDX = 0
VOCAB, K, E, H = 30000, 20, 256, 256
B, T = 128, 512
NCORES = 8
BL = B // NCORES          # 16 sequences per core
NG = 2                    # batch groups per direction
GB = BL // NG
SLOTW = 4                 # window length (steps); slot = 1 psum bank
NW = T // SLOTW           # 128 windows per direction
BLK = 32                  # emit/exp/unary block size
NBLK = T // BLK
RESCALE = 8               # CRF rescale interval (beta steps, per subgroup)
NSUB = 2                  # CRF column subgroups
SB = BL // NSUB
NCH = 64                  # 128-token gather chunks

_cache = {}


def _build_program():
    from contextlib import ExitStack
    import concourse.bass as bass
    import concourse.bacc as bacc
    import concourse.tile as tile
    from concourse import mybir
    from concourse.masks import make_identity

    f32 = mybir.dt.float32
    i32 = mybir.dt.int32
    bf16 = mybir.dt.bfloat16
    u8 = mybir.dt.uint8
    AF = mybir.ActivationFunctionType
    OP = mybir.AluOpType

    nc = bacc.Bacc(None, target_bir_lowering=False, debug=False)
    names = {}

    with ExitStack() as ctx:
        tc = ctx.enter_context(tile.TileContext(nc))
        dram = ctx.enter_context(tc.tile_pool(name="dram", bufs=1, space="DRAM"))

        def din(key, shape, dt=f32):
            t = dram.tile(shape, dt, kind="ExternalInput", name=key)
            names[key] = t.tensor.name
            return t

        emb = din("emb", [VOCAB, E], bf16)
        toks = din("toks", [T * BL, 1], i32)
        masku = din("masku", [1, T * BL], u8)
        negm = din("negm", [1, T * BL], bf16)
        tags1f = din("tags1f", [K, T * BL], u8)
        tagsnx = din("tagsnx", [T * BL, K], u8)
        tagsfl = din("tagsfl", [T * BL, 1], i32)
        wih = {d: din(f"wih_{d}", [E, 4 * H], bf16) for d in "fb"}
        whh = {d: din(f"whh_{d}", [E, 4 * H], bf16) for d in "fb"}
        brow = {d: din(f"brow_{d}", [1, 4 * H], bf16) for d in "fb"}
        woutT = din("woutT", [4, 128, K], bf16)
        bout = din("bout", [K, 1])
        transT = din("transT", [K, K])
        trans = din("trans", [K, K])
        out_loss = dram.tile([1, BL], f32, kind="ExternalOutput")
        names["out"] = out_loss.tensor.name
        out_dbg = dram.tile([1, 4 * BL], f32, kind="ExternalOutput", name="out_dbg")
        names["dbg"] = out_dbg.tensor.name

        # PSUM: 4 window slots (1 bank each) + transpose + small tiles
        ps_slot = ctx.enter_context(tc.tile_pool(name="ps_slot", bufs=1, space="PSUM"))
        ps_t = ctx.enter_context(tc.tile_pool(name="ps_t", bufs=1, space="PSUM"))
        ps_s = ctx.enter_context(tc.tile_pool(name="ps_s", bufs=2, space="PSUM"))

        sg = ctx.enter_context(tc.tile_pool(name="sg", bufs=1))
        tmp = ctx.enter_context(tc.tile_pool(name="tmp", bufs=4))
        gat = ctx.enter_context(tc.tile_pool(name="gat", bufs=4))
        neg = ctx.enter_context(tc.tile_pool(name="neg", bufs=2))
        fin = ctx.enter_context(tc.tile_pool(name="fin", bufs=3))

        slot = {d: [ps_slot.tile([128, 8, SLOTW, BL], f32, tag=f"slot{d}{i}",
                                 name=f"slot{d}{i}") for i in range(2)]
                for d in "fb"}

        # ---- resident SBUF tensors ----
        s_wih = {d: sg.tile([128, 2, 4 * H], bf16, tag=f"wih{d}", name=f"wih{d}")
                 for d in "fb"}
        s_whh = {d: sg.tile([128, 2, 4 * H], bf16, tag=f"whh{d}", name=f"whh{d}")
                 for d in "fb"}
        s_brow = {d: sg.tile([1, 4 * H], bf16, tag=f"brow{d}", name=f"brow{d}")
                  for d in "fb"}
        for d in "fb":
            nc.sync.dma_start(out=s_wih[d][:], in_=wih[d][:].rearrange("(k p) m -> p k m", p=128))
            nc.sync.dma_start(out=s_whh[d][:], in_=whh[d][:].rearrange("(k p) m -> p k m", p=128))
            nc.sync.dma_start(out=s_brow[d][:], in_=brow[d][:])
        s_wout = sg.tile([128, 4, K], bf16, tag="wout")
        nc.sync.dma_start(out=s_wout[:], in_=woutT[:].rearrange("c p k -> p c k"))
        s_bout = sg.tile([K, 1], f32, tag="bout")
        nc.sync.dma_start(out=s_bout[:], in_=bout[:])
        s_transT = sg.tile([K, K], f32, tag="transT")
        nc.sync.dma_start(out=s_transT[:], in_=transT[:])
        s_expAT = sg.tile([K, K], f32, tag="expAT")
        nc.scalar.activation(s_expAT[:], s_transT[:], AF.Exp)

        onesb = sg.tile([1, 256], bf16, tag="onesb")
        nc.vector.memset(onesb[:], 1.0)
        onesrow = sg.tile([1, 128], bf16, tag="onesrow")
        nc.vector.memset(onesrow[:], 1.0)
        onesf = sg.tile([128, K], f32, tag="onesf")
        nc.vector.memset(onesf[:], 1.0)
        ident = sg.tile([128, 128], bf16, tag="ident")
        make_identity(nc, ident[:])

        c_mask = sg.tile([1, 1], i32, tag="c_mask")
        nc.vector.memset(c_mask[:], 0x7F800000)

        maskrep = sg.tile([128, T, BL], u8, tag="maskrep")
        nc.sync.dma_start(
            out=maskrep[:],
            in_=bass.AP(tensor=masku.tensor, offset=masku[:].offset,
                        ap=[[0, 128], [BL, T], [1, BL]]))

        idxall = sg.tile([128, NCH], i32, tag="idxall")
        nc.sync.dma_start(out=idxall[:],
                          in_=bass.AP(tensor=toks.tensor, offset=toks[:].offset,
                                      ap=[[1, 128], [128, NCH]]))
        idxtag = sg.tile([128, NCH], i32, tag="idxtag")
        nc.sync.dma_start(out=idxtag[:],
                          in_=bass.AP(tensor=tagsfl.tensor, offset=tagsfl[:].offset,
                                      ap=[[1, 128], [128, NCH]]))
        s_t1f = sg.tile([K, T * BL], u8, tag="s_t1f")
        nc.sync.dma_start(out=s_t1f[:], in_=tags1f[:])
        s_tnx = sg.tile([128, NCH, K], u8, tag="s_tnx")
        nc.sync.dma_start(out=s_tnx[:],
                          in_=tagsnx[:].rearrange("(n p) k -> p n k", p=128))

        xT = sg.tile([128, 2, T * BL], bf16, tag="xT")
        emit = sg.tile([K, T, BL], f32, tag="emit")
        expE = sg.tile([K, 2, BLK * BL], f32, tag="expE")
        hist = {d: sg.tile([128, 2, T, BL], bf16, tag=f"hist{d}", name=f"hist{d}")
                for d in "fb"}
        hzero = sg.tile([128, 2, BL], bf16, tag="hzero")
        nc.vector.memset(hzero[:], 0.0)

        st_c = {d: sg.tile([128, 2, BL], f32, tag=f"c{d}", name=f"c{d}") for d in "fb"}
        for d in "fb":
            nc.vector.memset(st_c[d][:], 0.0)

        Bv = sg.tile([K, BL], f32, tag="Bv")
        nc.vector.memset(Bv[:], 1.0)
        Eacc = sg.tile([1, BL], f32, tag="Eacc")
        nc.vector.memset(Eacc[:], 0.0)
        Uacc = sg.tile([K, BL], f32, tag="Uacc")
        nc.vector.memset(Uacc[:], 0.0)
        TRbuf = sg.tile([128, NCH], f32, tag="TRbuf")

        # ---- warm-up matmuls ----
        for wt in [s_wih["f"][:, 0, 0:1], s_wih["b"][:, 0, 0:1],
                   s_whh["f"][:, 0, 0:1], s_whh["b"][:, 0, 0:1],
                   s_wout[:, 0, 0:1], ident[:, 0:1]]:
            psd = ps_s.tile([1, 1], f32, tag="pssm", name="psd")
            nc.tensor.matmul(psd[:], lhsT=wt, rhs=wt, start=True, stop=True)
        psd = ps_s.tile([1, 1], f32, tag="pssm", name="psd")
        nc.tensor.matmul(psd[:], lhsT=s_expAT[0:K, 0:1], rhs=s_expAT[0:K, 0:1],
                         start=True, stop=True)

        # ---------------- background work queue ----------------
        bg_q = []

        def drain_bg(n):
            while n > 0 and bg_q:
                bg_q.pop(0)()
                n -= 1

        gathered = [0]

        def gather_chunk(c):
            def work():
                xg = gat.tile([128, E], bf16, tag="xg", name="xg")
                nc.gpsimd.indirect_dma_start(
                    out=xg[:], out_offset=None, in_=emb[:],
                    in_offset=bass.IndirectOffsetOnAxis(ap=idxall[:, c:c + 1], axis=0))
                for k in range(2):
                    pst = ps_t.tile([128, 128], bf16, tag="pst", name="pst")
                    nc.tensor.transpose(out=pst[:], in_=xg[:, k * 128:(k + 1) * 128],
                                        identity=ident[:])
                    nc.vector.tensor_copy(xT[:, k, c * 128:(c + 1) * 128], pst[:])
            return work

        def tag_chunk(i):
            def work():
                tr = gat.tile([128, K], f32, tag="tr", name="tr")
                nc.gpsimd.indirect_dma_start(
                    out=tr[:], out_offset=None, in_=trans[:],
                    in_offset=bass.IndirectOffsetOnAxis(ap=idxtag[:, i:i + 1], axis=0))
                sel = gat.tile([128, K], f32, tag="sel", name="sel")
                nc.vector.tensor_copy(sel[:], s_tnx[:, i, :])
                nc.vector.tensor_tensor(tr[:], tr[:], sel[:], op=OP.mult)
                nc.vector.tensor_reduce(TRbuf[:, i:i + 1], tr[:],
                                        axis=mybir.AxisListType.X, op=OP.add)
            return work

        # ---------------- window build pieces ----------------
        def negdma_piece(w, nm):
            c0, ncol = w * SLOTW * BL, SLOTW * BL

            def work():
                nc.sync.dma_start(out=nm[:, 0:ncol], in_=negm[:, c0:c0 + ncol])
            return work

        def proj_piece(w, d, m0, nm):
            c0, ncol = w * SLOTW * BL, SLOTW * BL
            sl = slot[d][w % 2]

            def work():
                for m in (m0, m0 + 1):
                    o_m = bass.AP(tensor=sl.tensor,
                                  offset=sl[:].offset + m * SLOTW * BL,
                                  ap=[sl[:].ap[0], [1, ncol]])
                    for k in range(2):
                        nc.tensor.matmul(o_m, lhsT=s_wih[d][:, k, m * 128:(m + 1) * 128],
                                         rhs=xT[:, k, c0:c0 + ncol],
                                         start=(m == 0 and k == 0), stop=False,
                                         skip_group_check=True)
                    nc.tensor.matmul(o_m, lhsT=s_brow[d][:, m * 128:(m + 1) * 128],
                                     rhs=onesb[:, 0:ncol], start=False, stop=False,
                                     skip_group_check=True)
                    if d == "b" and m < 6:
                        nc.tensor.matmul(o_m, lhsT=onesrow[:, 0:128],
                                         rhs=nm[:, 0:ncol], start=False, stop=False,
                                         skip_group_check=True)
            return work

        def build_pieces(w, d):
            pieces = []
            nm = None
            if d == "b":
                nm = neg.tile([1, 256], bf16, tag="nm", name="nm")
                pieces.append(negdma_piece(w, nm))
            for m0 in range(0, 8, 2):
                pieces.append(proj_piece(w, d, m0, nm))
            return pieces

        # ---------------- LSTM step ----------------
        def lstm_step(d, t):
            w = t // SLOTW
            sl = slot[d][w % 2]
            toff = t - w * SLOTW
            c = st_c[d]
            tprev = t - 1 if d == "f" else t + 1
            hin = hzero if (d == "f" and t == 0) or (d == "b" and t == T - 1) \
                else None
            for m in range(8):
                o_m = bass.AP(tensor=sl.tensor,
                              offset=sl[:].offset + (m * SLOTW + toff) * BL,
                              ap=[sl[:].ap[0], [1, BL]])
                for k in range(2):
                    rhs = hin[:, k, :] if hin is not None else hist[d][:, k, tprev, :]
                    nc.tensor.matmul(o_m, lhsT=s_whh[d][:, k, m * 128:(m + 1) * 128],
                                     rhs=rhs, start=False, stop=False,
                                     skip_group_check=True)
            for g in range(NG):
                cs = slice(g * GB, (g + 1) * GB)
                gin = bass.AP(tensor=sl.tensor,
                              offset=sl[:].offset + toff * BL + g * GB,
                              ap=[sl[:].ap[0], [SLOTW * BL, 8], [1, GB]])
                s = tmp.tile([128, 8, GB], f32, tag=f"s{d}{g}", name=f"s{d}{g}")
                nc.scalar.activation(s[:], gin, AF.Sigmoid)
                si, sf, so, sgt = s[:, 0:2], s[:, 2:4], s[:, 4:6], s[:, 6:8]
                ig = tmp.tile([128, 2, GB], f32, tag=f"ig{d}{g}", name=f"ig{d}{g}")
                nc.vector.scalar_tensor_tensor(ig[:], sgt, 0.5, si,
                                               op0=OP.subtract, op1=OP.mult)
                fc = tmp.tile([128, 2, GB], f32, tag=f"fc{d}{g}", name=f"fc{d}{g}")
                nc.gpsimd.tensor_tensor(fc[:], sf, c[:, :, cs], op=OP.mult)
                nc.vector.scalar_tensor_tensor(c[:, :, cs], ig[:], 2.0, fc[:],
                                               op0=OP.mult, op1=OP.add)
                th = tmp.tile([128, 2, GB], f32, tag=f"th{d}{g}", name=f"th{d}{g}")
                nc.scalar.activation(th[:], c[:, :, cs], AF.Tanh)
                nc.vector.tensor_tensor(hist[d][:, :, t, cs], so, th[:], op=OP.mult)

        # ---------------- emission block assembly ----------------
        def emit_chunk(blk, q):
            """emit[:, blk*32+q*8 : .. +8, :] = woutF@hf + woutB@hb + bias."""
            t0 = blk * BLK + q * 8
            n = 8 * BL

            def work():
                pe = ps_s.tile([K, n], f32, tag="pssm", name="pe")
                for ci, d in ((0, "f"), (2, "b")):
                    for k in range(2):
                        nc.tensor.matmul(
                            pe[:], lhsT=s_wout[:, ci + k, :],
                            rhs=hist[d][:, k, t0:t0 + 8, :].rearrange("p t b -> p (t b)"),
                            start=(ci == 0 and k == 0), stop=(ci == 2 and k == 1))
                nc.scalar.activation(
                    emit[:, t0:t0 + 8, :].rearrange("k t b -> k (t b)"),
                    pe[:], AF.Identity, bias=s_bout[:, 0:1])
            return work

        def unary_piece(blk):
            c0 = blk * BLK * BL
            n = BLK * BL

            def work():
                src = emit[:, blk * BLK:(blk + 1) * BLK, :].rearrange("k t b -> k (t b)")
                t1 = tmp.tile([K, n], f32, tag="t1", name="t1", bufs=1)
                nc.scalar.activation(t1[:], s_t1f[:, c0:c0 + n], AF.Identity)
                um = tmp.tile([K, n], f32, tag="um", name="um", bufs=1)
                nc.gpsimd.tensor_tensor(um[:], t1[:], src, op=OP.mult)
                ur = tmp.tile([K, BL], f32, tag="ur", name="ur")
                umr = bass.AP(tensor=um.tensor, offset=um[:].offset,
                              ap=[um[:].ap[0], [1, BL], [BL, BLK]])
                nc.vector.tensor_reduce(ur[:], umr, axis=mybir.AxisListType.X, op=OP.add)
                nc.vector.tensor_tensor(Uacc[:], Uacc[:], ur[:], op=OP.add)
            return work

        # ---------------- CRF beta tail ----------------
        rescale_count = [0]

        def exp_block(blk):
            src = emit[:, blk * BLK:(blk + 1) * BLK, :].rearrange("k t b -> k (t b)")
            nc.scalar.activation(expE[:, blk % 2, :], src, AF.Exp)

        def beta_step(s, sub):
            cs = slice(sub * SB, (sub + 1) * SB)
            blk = (s + 1) // BLK
            col = ((s + 1) % BLK) * BL + sub * SB
            bp = tmp.tile([K, SB], f32, tag=f"bp{sub}", name=f"bp{sub}")
            nc.gpsimd.tensor_tensor(bp[:], Bv[:, cs], expE[:, blk % 2, col:col + SB],
                                    op=OP.mult)
            psb = ps_s.tile([K, SB], f32, tag="pssm", name="psb")
            nc.tensor.matmul(psb[:], lhsT=s_expAT[:], rhs=bp[:], start=True, stop=True)
            nc.vector.copy_predicated(Bv[:, cs], maskrep[0:K, s + 1, cs], psb[:])

        def beta_rescale(sub):
            cs = slice(sub * SB, (sub + 1) * SB)
            pss = ps_s.tile([1, SB], f32, tag="pssm", name="pssr")
            nc.tensor.matmul(pss[:], lhsT=onesf[0:K, 0:1], rhs=Bv[:, cs],
                             start=True, stop=True)
            em = tmp.tile([1, SB], i32, tag=f"em{sub}", name=f"em{sub}")
            nc.vector.tensor_scalar(em[:], pss[:].bitcast(i32), c_mask[:, 0:1],
                                    None, op0=OP.bitwise_and)
            ef = tmp.tile([1, SB], f32, tag=f"ef{sub}", name=f"ef{sub}")
            nc.vector.tensor_copy(ef[:], em[:])
            nc.vector.scalar_tensor_tensor(Eacc[:, cs], ef[:], 1.0 / (1 << 23),
                                           Eacc[:, cs], op0=OP.mult, op1=OP.add)
            scf = tmp.tile([1, SB], f32, tag=f"scf{sub}", name=f"scf{sub}")
            nc.vector.tensor_scalar(scf[:], ef[:], -1.0, float(0x7F000000),
                                    op0=OP.mult, op1=OP.add)
            sci = tmp.tile([1, SB], i32, tag=f"sci{sub}", name=f"sci{sub}")
            nc.vector.tensor_copy(sci[:], scf[:])
            psr = ps_s.tile([K, SB], f32, tag="pssm", name="psr")
            nc.tensor.matmul(psr[:], lhsT=onesf[0:1, 0:K], rhs=sci[:].bitcast(f32),
                             start=True, stop=True)
            nc.vector.tensor_tensor(Bv[:, cs], Bv[:, cs], psr[:], op=OP.mult)
            rescale_count[0] += 1

        # ================ merged LSTM phase ================
        misc_q = []

        def drain_misc(n):
            while n > 0 and misc_q:
                misc_q.pop(0)()
                n -= 1

        # token gather cursors: fwd consumes chunks ascending, bwd descending
        lo, hi = [0], [NCH - 1]

        def need_lo(upto_col):
            while lo[0] * 128 < upto_col and lo[0] <= hi[0]:
                bg_q.append(gather_chunk(lo[0]))
                lo[0] += 1

        def need_hi(from_col):
            while (hi[0] + 1) * 128 > from_col and hi[0] >= lo[0]:
                bg_q.append(gather_chunk(hi[0]))
                hi[0] -= 1

        # block assembly readiness: blk fully covered at merged step
        # s >= max(blk*BLK+BLK-1, T-1-blk*BLK)
        ready_at = {}
        for blk in range(NBLK):
            ready_at.setdefault(max(blk * BLK + BLK - 1, T - 1 - blk * BLK),
                                []).append(blk)

        # prime: tokens + first windows of both directions
        need_lo(SLOTW * BL)
        need_hi((NW - 1) * SLOTW * BL)
        for p in build_pieces(0, "f"):
            bg_q.append(p)
        for p in build_pieces(NW - 1, "b"):
            bg_q.append(p)
        drain_bg(len(bg_q))

        tag_i = [0]
        for s in range(T):
            tf, tb = s, T - 1 - s
            wf, wb = tf // SLOTW, tb // SLOTW
            if s % SLOTW == 0:
                if wf + 1 < NW:
                    need_lo((wf + 2) * SLOTW * BL)
                    for p in build_pieces(wf + 1, "f"):
                        bg_q.append(p)
                if wb - 1 >= 0:
                    need_hi((wb - 1) * SLOTW * BL)
                    for p in build_pieces(wb - 1, "b"):
                        bg_q.append(p)
            if s % 8 == 0 and tag_i[0] < NCH:
                misc_q.append(tag_chunk(tag_i[0]))
                tag_i[0] += 1
            lstm_step("f", tf)
            lstm_step("b", tb)
            for blk in ready_at.get(s, []):
                for q in range(4):
                    misc_q.append(emit_chunk(blk, q))
                misc_q.append(unary_piece(blk))
            drain_bg(3)
            drain_misc(1)
        drain_bg(len(bg_q))
        drain_misc(len(misc_q))

        # ================ CRF beta tail ================
        exp_block(NBLK - 1)
        for s in range(T - 2, -1, -1):
            if (s + 1) % BLK == BLK - 1:
                exp_block((s + 1) // BLK)
            for sub in range(NSUB):
                beta_step(s, sub)
            if s % RESCALE == 0 and s > 0:
                for sub in range(NSUB):
                    beta_rescale(sub)

        # ================ finalize ================
        zt = fin.tile([K, BL], f32, tag="zt")
        nc.vector.tensor_tensor(zt[:], Bv[:], expE[:, 0, 0:BL], op=OP.mult)
        psz = ps_s.tile([1, BL], f32, tag="pssm", name="psz")
        nc.tensor.matmul(psz[:], lhsT=onesf[0:K, 0:1], rhs=zt[:], start=True, stop=True)
        logZ = fin.tile([1, BL], f32, tag="logZ")
        nc.scalar.activation(logZ[:], psz[:], AF.Ln)
        nc.vector.scalar_tensor_tensor(logZ[:], Eacc[:], float(np.log(2.0)), logZ[:],
                                       op0=OP.mult, op1=OP.add)
        nc.vector.tensor_scalar(
            logZ[:], logZ[:],
            float(-127.0 * (rescale_count[0] // NSUB) * np.log(2.0)), None,
            op0=OP.add)

        psu = ps_s.tile([1, BL], f32, tag="pssm", name="psu")
        nc.tensor.matmul(psu[:], lhsT=onesf[0:K, 0:1], rhs=Uacc[:], start=True, stop=True)
        score = fin.tile([1, BL], f32, tag="score")
        nc.vector.tensor_copy(score[:], psu[:])

        QT = T // 128
        pstr = ps_s.tile([1, NCH], f32, tag="pssm", name="pstr")
        nc.tensor.matmul(pstr[:], lhsT=onesf[:, 0:1], rhs=TRbuf[:], start=True, stop=True)
        trv = fin.tile([1, BL], f32, tag="trv")
        ptr_ap = bass.AP(tensor=pstr.tensor, offset=pstr[:].offset,
                         ap=[pstr[:].ap[0], [QT, BL], [1, QT]])
        nc.vector.tensor_reduce(trv[:], ptr_ap, axis=mybir.AxisListType.X, op=OP.add)

        dbg = fin.tile([1, 4 * BL], f32, tag="dbg")
        nc.vector.tensor_copy(dbg[:, 0 * BL:1 * BL], score[:])
        nc.vector.tensor_copy(dbg[:, 1 * BL:2 * BL], trv[:])
        nc.vector.tensor_copy(dbg[:, 2 * BL:3 * BL], logZ[:])
        nc.vector.tensor_copy(dbg[:, 3 * BL:4 * BL], Eacc[:])
        nc.sync.dma_start(out=out_dbg[:], in_=dbg[:])

        nc.vector.tensor_tensor(score[:], score[:], trv[:], op=OP.add)
        res = fin.tile([1, BL], f32, tag="res")
        nc.vector.tensor_tensor(res[:], logZ[:], score[:], op=OP.subtract)
        nc.sync.dma_start(out=out_loss[:], in_=res[:])

    nc.compile()
    return nc, names


def _prep_core(inputs, core, perm):
    import ml_dtypes
    bf = ml_dtypes.bfloat16
    s = slice(core * BL, (core + 1) * BL)
    sent = np.asarray(inputs["sentences"][s])
    tags = np.asarray(inputs["tags"][s])
    mask = (sent != PAD_IDX)
    maskT = mask.T
    toks = np.ascontiguousarray(sent.T).reshape(T * BL, 1)
    oh = (tags[:, :, None] == np.arange(K)[None, None, :])
    tags1h = (oh & mask[:, :, None]).transpose(2, 1, 0).reshape(K, T * BL)
    tnx = np.zeros((BL, T, K), np.float32)
    tnx[:, :-1, :] = (oh[:, 1:, :] & mask[:, 1:, None]).astype(np.float32)

    def wprep(wname):
        wt = np.asarray(inputs[wname], np.float32)[perm].copy()
        wt[6 * 128:, :] *= 2.0
        return np.ascontiguousarray(wt.T).astype(bf)

    bvec = {}
    for d, key in (("f", "b_f"), ("b", "b_b")):
        bb = np.asarray(inputs[key], np.float32)[perm].copy()
        bb[6 * 128:] *= 2.0
        bvec[d] = bb.reshape(1, 4 * H).astype(bf)

    return {
        "toks": toks.astype(np.int32),
        "masku": maskT.astype(np.uint8).reshape(1, T * BL),
        "negm": ((~maskT).astype(np.float32) * -1e5).reshape(1, T * BL).astype(bf),
        "tags1f": tags1h.astype(np.uint8),
        "tagsnx": tnx.reshape(T * BL, K).astype(np.uint8),
        "tagsfl": tags.reshape(T * BL, 1).astype(np.int32),
        "emb": np.asarray(inputs["embedding"], np.float32).astype(bf),
        "wih_f": wprep("w_ih_f"), "wih_b": wprep("w_ih_b"),
        "whh_f": wprep("w_hh_f"), "whh_b": wprep("w_hh_b"),
        "brow_f": bvec["f"], "brow_b": bvec["b"],
        "woutT": np.ascontiguousarray(
            np.asarray(inputs["w_out"], np.float32).T.reshape(4, 128, K)).astype(bf),
        "bout": np.asarray(inputs["b_out"]).reshape(K, 1).astype(np.float32),
        "transT": np.ascontiguousarray(np.asarray(inputs["transition"]).T).astype(np.float32),
        "trans": np.asarray(inputs["transition"], np.float32),
    }


def kernel(**inputs):
    from concourse.bass_utils import run_bass_kernel_spmd

    if "prog" not in _cache:
        _cache["prog"] = _build_program()
    nc, names = _cache["prog"]

    blocks = np.arange(4 * H).reshape(4, H)
    perm = np.concatenate([blocks[0], blocks[1], blocks[3], blocks[2]])

    in_maps = []
    for core in range(NCORES):
        m = _prep_core(inputs, core, perm)
        in_maps.append({names[kk]: vv for kk, vv in m.items()})

    res = run_bass_kernel_spmd(nc, in_maps, core_ids=list(range(NCORES)),
                               **_cache.get("run_kwargs", {}))
    out = np.concatenate([r[names["out"]].reshape(BL) for r in res.results])
    _cache["last_results"] = res
    if "dbg" in names:
        _cache["dbg"] = np.concatenate(
            [r[names["dbg"]].reshape(4, BL) for r in res.results], axis=1)
    return out.astype(np.float32)
